# revision 20
# baseline (speedup 1.0000x reference)
"""Block-sparse linear kernel for Trainium2 (8 NeuronCores, Bass/Tile).

Computes out = x @ (weight*mask).T + bias for
  x [4, 2048, 4096] f32, weight [4096, 4096] f32, mask [4096,4096] bool,
  bias [4096] f32  ->  out [4, 2048, 4096] f32.

Strategy (data-parallel over tokens, 8 cores x 1024 tokens each):
  The 16x16 block mask is coarsened into 32x32 cells (2 input blocks x
  2 output blocks) by overlap-maximizing pairing on both dimensions.
  Only nonzero cells are computed, as [32,32,512] PE-tiled fp16 matmuls
  (fp32 accumulate in PSUM) on 16 concurrent tensor-engine sub-arrays
  (4 row groups x 4 col groups).  ~4480 cells vs 16384 dense.

  Input pairs are assigned to 4 partition classes (32 each); output
  pairs to 32 sets of 4 (one per col group), balanced so each set's
  16 slot lists have near-equal length.  Two token phases of 512; per
  (phase, set) the 4 class-banks accumulate slots, then drain as
  ACT (bias add) + 2 VectorE adds + 1 GpSimd add, overlapped.

  Redundant same-engine dependency edges are pruned before semaphore
  assignment so only group-final matmuls carry semaphore increments
  (engines complete instructions in program order), lifting the
  tensor-queue issue rate from ~34ns to ~9-13ns per LDW+MM pair.

  Falls back to a dense fp16 kernel when the mask is not sparse enough.
"""

import sys

for _p in ("/opt/trn_rl_repo",):
    if _p not in sys.path:
        sys.path.insert(0, _p)

import time

import numpy as np

import concourse.bacc as bacc
import concourse.mybir as mybir
import concourse.tile as tile
import concourse.tile_sem_assignment as _tsa
from concourse import bass_utils

P = 128
IN = 4096
OUT = 4096
BLK = 16
NB = IN // BLK        # 256 blocks per dim
NPAIR = NB // 2       # 128 pairs per dim
M = 32                # out-features per cell
NSET = 32             # sets of 4 output pairs
KO = NPAIR // 4       # 32 ko slots per class
N_CORES = 8
TOK = 1024
NCHUNK = 512
NT = TOK // NCHUNK    # 2
F16 = mybir.dt.float16
F32 = mybir.dt.float32

SPARSE_MAX_CELLS = 9000


# ------------------------------------------------------- dependency pruning
# Engines complete instructions in program order (PE matmuls are
# pc-monotone in start and end; ACT/DVE queues are strict FIFO), so a
# consumer depending on several producers from one engine only needs the
# latest edge.  Tile emits one edge per tile access, making every matmul
# carry a serialized ~26ns semaphore increment; prune to group-finals.
# GpSimd (8 Q7 cores) and DMAs (multiple queues) are NOT order-guaranteed.

_PRUNABLE = {
    ("Matmult", mybir.EngineType.PE),
    ("Activation", mybir.EngineType.Activation),
    ("Activation", mybir.EngineType.DVE),
    ("TensorTensor", mybir.EngineType.DVE),
    ("TensorScalarPtr", mybir.EngineType.DVE),
    ("TensorCopy", mybir.EngineType.DVE),
    ("Memset", mybir.EngineType.DVE),
}

_pruned_ids = set()


def _prune_blocks(blocks):
    if id(blocks) in _pruned_ids:
        return
    _pruned_ids.add(id(blocks))
    for bb_name, insts in blocks.items():
        order = {}
        by_name = {}
        for k, inst in enumerate(insts):
            order[inst.name] = k
            by_name[inst.name] = inst
        for d in insts:
            deps = d.sync_dependency_names() or ()
            if len(deps) < 2:
                continue
            groups = {}
            for pname in deps:
                p = by_name.get(pname)
                if p is None:
                    continue
                if (p.opcode, p.engine) in _PRUNABLE:
                    groups.setdefault(p.engine, []).append(p)
            for eng, plist in groups.items():
                if len(plist) < 2:
                    continue
                plist.sort(key=lambda p: order[p.name])
                for p in plist[:-1]:
                    d.remove_dependency(p.name)
                    if p.descendants is not None:
                        p.descendants.discard(d.name)


_orig_assign_ticks = _tsa.TileClockTick.assign_ticks
_install_done = False


def _install_prune():
    global _install_done
    if _install_done:
        return
    _install_done = True

    def assign_ticks_pruned(self, bb_name):
        _prune_blocks(self.ordered_instructions_by_block)
        return _orig_assign_ticks(self, bb_name)

    _tsa.TileClockTick.assign_ticks = assign_ticks_pruned


# ---------------------------------------------------------------- matching


def greedy_pair(support):
    """support: [N, D] bool rows. Pair rows maximizing overlap; [N/2, 2]."""
    N = support.shape[0]
    A = support.astype(np.int32)
    O = A @ A.T
    np.fill_diagonal(O, -1)
    pairs = []
    for _ in range(N // 2):
        idx = int(np.argmax(O))
        i, j = divmod(idx, N)
        pairs.append((i, j))
        O[i, :] = -1
        O[:, i] = -1
        O[j, :] = -1
        O[:, j] = -1
    return np.array(pairs, dtype=np.int64)


def cells_mat(bm, ip, op):
    u = bm[:, ip[:, 0]] | bm[:, ip[:, 1]]      # [256 outblk, 128 ipair]
    v = u[op[:, 0]] | u[op[:, 1]]              # [128 opair, 128 ipair]
    return v


def pair_opt(bm, ip, op, iters=60000, rng=None):
    """2-opt swaps of block membership in pairs, both dims."""
    rng = rng or np.random.default_rng(3)
    ip = ip.copy()
    op = op.copy()
    bq = bm[op[:, 0]] | bm[op[:, 1]]           # [128 opair, 256 inblk]
    bp = bm[:, ip[:, 0]] | bm[:, ip[:, 1]]     # [256 outblk, 128 ipair]
    for _ in range(iters):
        if rng.random() < 0.5:
            p1, p2 = rng.integers(0, NPAIR, 2)
            if p1 == p2:
                continue
            s1, s2 = rng.integers(0, 2, 2)
            a1, b1 = ip[p1, s1], ip[p1, 1 - s1]
            a2, b2 = ip[p2, s2], ip[p2, 1 - s2]
            old = ((bq[:, a1] | bq[:, b1]).sum()
                   + (bq[:, a2] | bq[:, b2]).sum())
            new = ((bq[:, a2] | bq[:, b1]).sum()
                   + (bq[:, a1] | bq[:, b2]).sum())
            if new <= old:
                ip[p1, s1], ip[p2, s2] = a2, a1
                bp[:, p1] = bm[:, ip[p1, 0]] | bm[:, ip[p1, 1]]
                bp[:, p2] = bm[:, ip[p2, 0]] | bm[:, ip[p2, 1]]
        else:
            p1, p2 = rng.integers(0, NPAIR, 2)
            if p1 == p2:
                continue
            s1, s2 = rng.integers(0, 2, 2)
            a1, b1 = op[p1, s1], op[p1, 1 - s1]
            a2, b2 = op[p2, s2], op[p2, 1 - s2]
            old = ((bp[a1] | bp[b1]).sum() + (bp[a2] | bp[b2]).sum())
            new = ((bp[a2] | bp[b1]).sum() + (bp[a1] | bp[b2]).sum())
            if new <= old:
                op[p1, s1], op[p2, s2] = a2, a1
                bq[p1] = bm[op[p1, 0]] | bm[op[p1, 1]]
                bq[p2] = bm[op[p2, 0]] | bm[op[p2, 1]]
    return ip, op


def balance(v, iters=120000, rng=None):
    """in-pairs -> 4 classes (32 each), out-pairs -> 32 sets of 4.
    Minimize sum over sets of the max slot count."""
    rng = rng or np.random.default_rng(5)
    cls = np.arange(NPAIR) % 4
    n = np.zeros((NPAIR, 4), dtype=np.int32)
    for r in range(4):
        n[:, r] = v[:, cls == r].sum(axis=1)
    key = n.max(axis=1)
    order = np.argsort(-key)
    setof = np.empty(NPAIR, dtype=np.int64)
    for s in range(NSET):
        setof[order[s * 4:(s + 1) * 4]] = s
    smax = np.array([n[setof == s].max() for s in range(NSET)], dtype=np.int64)
    c = int(smax.sum())
    vi = v.astype(np.int32)
    for _ in range(iters):
        if rng.random() < 0.5:
            a, b = rng.integers(0, NPAIR, 2)
            sa, sb = setof[a], setof[b]
            if sa == sb:
                continue
            setof[a], setof[b] = sb, sa
            na = n[setof == sa].max()
            nb = n[setof == sb].max()
            d = na + nb - smax[sa] - smax[sb]
            if d <= 0:
                smax[sa], smax[sb] = na, nb
                c += int(d)
            else:
                setof[a], setof[b] = sa, sb
        else:
            i, j = rng.integers(0, NPAIR, 2)
            r1, r2 = cls[i], cls[j]
            if r1 == r2:
                continue
            di = vi[:, i]
            dj = vi[:, j]
            n[:, r1] += dj - di
            n[:, r2] += di - dj
            news = np.array([n[setof == s].max() for s in range(NSET)],
                            dtype=np.int64)
            d = int(news.sum()) - c
            if d <= 0:
                smax = news
                c += d
                cls[i], cls[j] = r2, r1
            else:
                n[:, r1] -= dj - di
                n[:, r2] -= di - dj
    return cls, setof


def analyze_mask(mask):
    bm = mask.reshape(NB, BLK, NB, BLK).any(axis=(1, 3))  # [out_blk, in_blk]
    ip = greedy_pair(bm.T)
    best = None
    for _ in range(4):
        u = bm[:, ip[:, 0]] | bm[:, ip[:, 1]]
        op = greedy_pair(u)
        cand = int(cells_mat(bm, ip, op).sum())
        if best is None or cand < best[0]:
            best = (cand, ip.copy(), op.copy())
        w = bm[op[:, 0]] | bm[op[:, 1]]
        ip = greedy_pair(w.T)
        cand = int(cells_mat(bm, ip, op).sum())
        if cand < best[0]:
            best = (cand, ip.copy(), op.copy())
    _, ip, op = best
    ip, op = pair_opt(bm, ip, op)
    v = cells_mat(bm, ip, op)
    return ip, op, v


def build_schedule(v):
    """Returns (cls, setof, slot_lists, n_sr).

    slot_lists[s][r][c]: list of in-pair ids for set s, class r, slice c.
    """
    cls, setof = balance(v)
    # order pairs within each class -> ko index
    koof = np.empty(NPAIR, dtype=np.int64)
    for r in range(4):
        ids = np.nonzero(cls == r)[0]
        koof[ids] = np.arange(len(ids))
    set_pairs = [list(np.nonzero(setof == s)[0]) for s in range(NSET)]
    slot_lists = []
    for s in range(NSET):
        rows = []
        for r in range(4):
            colslots = []
            for c in range(4):
                po = set_pairs[s][c]
                ids = [int(i) for i in np.nonzero(v[po])[0] if cls[i] == r]
                ids.sort(key=lambda i: koof[i])
                colslots.append(ids)
            rows.append(colslots)
        slot_lists.append(rows)
    n_sr = np.zeros((NSET, 4), dtype=np.int64)
    for s in range(NSET):
        for r in range(4):
            n_sr[s, r] = sum(max(1, len(slot_lists[s][r][c]))
                             for c in range(4))
    return cls, setof, set_pairs, koof, slot_lists, n_sr


# ---------------------------------------------------------------- device


def build_sparse(slot_lists, n_sr, koof, dt=F16):
    _install_prune()
    nc = bacc.Bacc("TRN2", target_bir_lowering=False, debug=False)

    wsz = n_sr.max(axis=1)          # per-set uniform class width
    maxn = int(wsz.max())
    w_offs = np.zeros(NSET, dtype=np.int64)
    off = 0
    for s in range(NSET):
        w_offs[s] = off
        off += P * int(wsz[s]) * M
    w_words = int(off)

    xT = nc.dram_tensor("xT", [NT, P, KO, NCHUNK], dt, kind="ExternalInput")
    w = nc.dram_tensor("w", [w_words], dt, kind="ExternalInput")
    bias = nc.dram_tensor("bias", [P, NSET], F32, kind="ExternalInput")
    outT = nc.dram_tensor("outT", [NSET, P, TOK], F16, kind="ExternalOutput")

    with tile.TileContext(nc) as tc:
        with (
            tc.tile_pool(name="x_pool", bufs=1) as x_pool,
            tc.tile_pool(name="const", bufs=1) as const_pool,
            tc.tile_pool(name="w_pool", bufs=1) as w_pool,
            tc.tile_pool(name="drain_pool", bufs=4) as drain_pool,
            tc.tile_pool(name="out_pool", bufs=6) as out_pool,
            tc.tile_pool(name="psum", bufs=1, space="PSUM") as psum_pool,
        ):
            x0 = x_pool.tile([P, KO, NCHUNK], dt, name="x0")
            x1 = x_pool.tile([P, KO, NCHUNK], dt, name="x1")
            xts = [x0, x1]

            def emit_x_dma(n, g, ng=4):
                ks = KO // ng
                nc.sync.dma_start(
                    xts[n][:, g * ks:(g + 1) * ks],
                    xT.ap()[n, :, g * ks:(g + 1) * ks],
                )

            for g in range(4):
                emit_x_dma(0, g)
            bt = const_pool.tile([P, NSET], F32, name="bt")
            nc.sync.dma_start(bt[:], bias.ap())

            def emit_w_dma(s, wt):
                nwords = P * int(wsz[s]) * M
                o = int(w_offs[s])
                src = w.ap()[o:o + nwords].rearrange("(p f) -> p f", p=P)
                nc.sync.dma_start(wt[:, :int(wsz[s]) * M], src)

            # all weight tiles resident; weights DMA'd exactly once
            wts = []
            for s in range(NSET):
                wt = w_pool.tile([P, int(wsz[s]) * M], dt, tag=f"w{s}",
                                 name="wt")
                emit_w_dma(s, wt)
                if s == 3:
                    for g in range(4):
                        emit_x_dma(1, g)
                wts.append(wt)

            for n in range(NT):
                for s in range(NSET):
                    wt = wts[s]
                    par = s % 2
                    ps = [psum_pool.tile([P, NCHUNK], F32,
                                         tag=f"ps_{par}_{r}",
                                         name=f"ps_{par}_{r}")
                          for r in range(4)]

                    items = {}
                    widx = {}
                    for r in range(4):
                        k = 0
                        for c in range(4):
                            lst = slot_lists[s][r][c]
                            items[(r, c)] = lst if lst else [-1]
                            widx[(r, c)] = k
                            k += max(1, len(lst))
                    nsteps = max(len(items[(r, c)])
                                 for r in range(4) for c in range(4))
                    for st in range(nsteps):
                        for r in range(4):
                            for c in range(4):
                                lst = items[(r, c)]
                                if st >= len(lst):
                                    continue
                                i = lst[st]
                                wi = widx[(r, c)] + st
                                lhsT = wt[32 * r:32 * r + 32,
                                          wi * M:(wi + 1) * M]
                                ko = 0 if i < 0 else int(koof[i])
                                rhs = xts[n][32 * r:32 * r + 32,
                                             ko, :]
                                nc.tensor.matmul(
                                    ps[r][M * c:M * c + M, :], lhsT, rhs,
                                    start=(st == 0),
                                    stop=(st == len(lst) - 1),
                                    tile_position=(32 * r, M * c),
                                )
                    at = drain_pool.tile([P, NCHUNK], F32, tag="a",
                                         name="at")
                    nc.scalar.activation(
                        at[:], ps[0][:],
                        mybir.ActivationFunctionType.Identity,
                        bias=bt[:, s:s + 1],
                    )
                    bt2 = drain_pool.tile([P, NCHUNK], F32, tag="b",
                                          name="bt2")
                    nc.scalar.copy(bt2[:], ps[3][:])
                    ut = drain_pool.tile([P, NCHUNK], F32, tag="u",
                                         name="ut")
                    nc.vector.tensor_tensor(ut[:], ps[1][:], at[:],
                                            mybir.AluOpType.add)
                    nc.vector.tensor_tensor(ut[:], ut[:], ps[2][:],
                                            mybir.AluOpType.add)
                    ot = out_pool.tile([P, NCHUNK], F16, tag="ot",
                                       name="ot")
                    nc.gpsimd.tensor_tensor(ot[:], bt2[:], ut[:],
                                            mybir.AluOpType.add)
                    nc.sync.dma_start(
                        outT.ap()[s, :, n * NCHUNK:(n + 1) * NCHUNK],
                        ot[:])
    nc.compile()
    return nc


def build_dense(dt=F16):
    """Dense fallback: [128,128,512] matmuls, K-contiguous."""
    NM = OUT // P
    KOD = IN // P
    nc = bacc.Bacc("TRN2", target_bir_lowering=False, debug=False)
    xT = nc.dram_tensor("xT", [P, KOD, TOK], dt, kind="ExternalInput")
    w = nc.dram_tensor("w", [NM, P, KOD, P], dt, kind="ExternalInput")
    bias = nc.dram_tensor("bias", [P, NM], F32, kind="ExternalInput")
    outT = nc.dram_tensor("outT", [NM, P, TOK], F32, kind="ExternalOutput")

    with tile.TileContext(nc) as tc:
        with (
            tc.tile_pool(name="x_pool", bufs=1) as x_pool,
            tc.tile_pool(name="const", bufs=1) as const_pool,
            tc.tile_pool(name="w_pool", bufs=3) as w_pool,
            tc.tile_pool(name="out_pool", bufs=4) as out_pool,
            tc.tile_pool(name="psum", bufs=2, space="PSUM") as psum_pool,
        ):
            xt = x_pool.tile([P, KOD, TOK], dt, name="xt")
            nc.sync.dma_start(xt[:], xT.ap())
            bt = const_pool.tile([P, NM], F32, name="bt")
            nc.sync.dma_start(bt[:], bias.ap())
            for m in range(NM):
                wt = w_pool.tile([P, KOD, P], dt, name="wt")
                nc.sync.dma_start(wt[:], w.ap()[m])
                for n in range(NT):
                    psd = psum_pool.tile([P, NCHUNK], F32, name="psd")
                    for ko in range(KOD):
                        nc.tensor.matmul(
                            psd[:], wt[:, ko],
                            xt[:, ko, n * NCHUNK:(n + 1) * NCHUNK],
                            start=(ko == 0), stop=(ko == KOD - 1),
                        )
                    ot = out_pool.tile([P, NCHUNK], F32, name="ot")
                    nc.scalar.activation(
                        ot[:], psd[:], mybir.ActivationFunctionType.Identity,
                        bias=bt[:, m:m + 1],
                    )
                    nc.sync.dma_start(
                        outT.ap()[m, :, n * NCHUNK:(n + 1) * NCHUNK], ot[:]
                    )
    nc.compile()
    return nc


# ---------------------------------------------------------------- packing


def pair_feats(pairs, p):
    a, b = pairs[p]
    return np.concatenate([np.arange(a * BLK, (a + 1) * BLK),
                           np.arange(b * BLK, (b + 1) * BLK)])


def pack_weights(weight, mask, ip, op, set_pairs, slot_lists, n_sr):
    wm = weight.astype(np.float32) * mask
    wsz = n_sr.max(axis=1)
    total = int(P * (wsz * M).sum())
    out = np.zeros(total, dtype=np.float32)
    off = 0
    for s in range(NSET):
        nn = int(wsz[s])
        blockbuf = np.zeros((P, nn * M), dtype=np.float32)
        for r in range(4):
            k = 0
            for c in range(4):
                po = set_pairs[s][c]
                ofeat = pair_feats(op, po)
                lst = slot_lists[s][r][c]
                if not lst:
                    k += 1
                    continue
                for i in lst:
                    ifeat = pair_feats(ip, i)
                    blockbuf[32 * r:32 * r + 32, k * M:(k + 1) * M] = \
                        wm[np.ix_(ofeat, ifeat)].T
                    k += 1
        nw = P * nn * M
        out[off:off + nw] = blockbuf.reshape(-1)
        off += nw
    return out.astype(np.float16)


def pack_x_shard(x_shard, ip, cls, koof):
    src_feat = np.empty((P, KO), dtype=np.int64)
    for i in range(NPAIR):
        r, k = int(cls[i]), int(koof[i])
        src_feat[32 * r:32 * r + 32, k] = pair_feats(ip, i)
    xs = x_shard.astype(np.float16)           # [TOK, IN]
    xt = xs.T[src_feat.reshape(-1)].reshape(P, KO, TOK)
    # -> [NT, P, KO, NCHUNK]
    xt = xt.reshape(P, KO, NT, NCHUNK).transpose(2, 0, 1, 3)
    return np.ascontiguousarray(xt)


def pack_bias(bias, op, set_pairs):
    bp = np.zeros((P, NSET), dtype=np.float32)
    b = bias.astype(np.float32)
    for s in range(NSET):
        for c in range(4):
            bp[M * c:M * c + M, s] = b[pair_feats(op, set_pairs[s][c])]
    return bp


def out_feat_map(op, set_pairs):
    m = np.empty(OUT, dtype=np.int64)
    for s in range(NSET):
        for c in range(4):
            m[s * P + M * c:s * P + M * c + M] = pair_feats(
                op, set_pairs[s][c])
    return m


# ---------------------------------------------------------------- entry

_CACHE = {}


def _run_sparse(x, weight, bias, mask, plan):
    (nc, ip, op, cls, koof, set_pairs, slot_lists, n_sr) = plan
    w_flat = pack_weights(weight, mask, ip, op, set_pairs, slot_lists, n_sr)
    bias_p = pack_bias(bias, op, set_pairs)
    B, S = x.shape[0], x.shape[1]
    xf = np.ascontiguousarray(x.reshape(B * S, IN))
    in_maps = []
    for cidx in range(N_CORES):
        xs = xf[cidx * TOK:(cidx + 1) * TOK]
        in_maps.append({"xT": pack_x_shard(xs, ip, cls, koof), "w": w_flat,
                        "bias": bias_p})
    res = bass_utils.run_bass_kernel_spmd(
        nc, in_maps, core_ids=list(range(N_CORES)))
    fmap = out_feat_map(op, set_pairs)
    outs = []
    for cidx in range(N_CORES):
        o = res.results[cidx]["outT"].reshape(OUT, TOK).astype(np.float32)
        unperm = np.empty_like(o)
        unperm[fmap] = o
        outs.append(unperm.T)
    full = np.concatenate(outs, axis=0)
    return np.ascontiguousarray(full.reshape(B, S, OUT))


def _run_dense(x, weight, bias, mask, nc):
    NM = OUT // P
    KOD = IN // P
    wm = (weight.astype(np.float32) * mask).astype(np.float16)
    w_packed = np.ascontiguousarray(
        wm.T.reshape(KOD, P, NM, P).transpose(2, 1, 0, 3))
    bias_p = np.ascontiguousarray(bias.astype(np.float32).reshape(NM, P).T)
    B, S = x.shape[0], x.shape[1]
    xf = np.ascontiguousarray(x.reshape(B * S, IN))
    in_maps = []
    for cidx in range(N_CORES):
        xs = xf[cidx * TOK:(cidx + 1) * TOK].astype(np.float16)
        xp = np.ascontiguousarray(xs.T.reshape(KOD, P, TOK).transpose(1, 0, 2))
        in_maps.append({"xT": xp, "w": w_packed, "bias": bias_p})
    res = bass_utils.run_bass_kernel_spmd(
        nc, in_maps, core_ids=list(range(N_CORES)))
    outs = []
    for cidx in range(N_CORES):
        o = res.results[cidx]["outT"].reshape(OUT, TOK)
        outs.append(o.T)
    full = np.concatenate(outs, axis=0)
    return np.ascontiguousarray(full.reshape(B, S, OUT).astype(np.float32))


def kernel(x, weight, bias, mask):
    x = np.asarray(x, dtype=np.float32)
    weight = np.asarray(weight, dtype=np.float32)
    bias = np.asarray(bias, dtype=np.float32)
    mask = np.asarray(mask).astype(bool)
    assert x.shape == (4, 2048, IN) and weight.shape == (OUT, IN)

    key = hash(mask.tobytes())
    if key not in _CACHE:
        ip, op, v = analyze_mask(mask)
        cells = int(v.sum())
        if cells <= SPARSE_MAX_CELLS:
            cls, setof, set_pairs, koof, slot_lists, n_sr = build_schedule(v)
            nc = build_sparse(slot_lists, n_sr, koof)
            _CACHE[key] = ("sparse", (nc, ip, op, cls, koof, set_pairs,
                                      slot_lists, n_sr))
        else:
            _CACHE[key] = ("dense", build_dense())
    kind, plan = _CACHE[key]
    if kind == "sparse":
        return _run_sparse(x, weight, bias, mask, plan)
    return _run_dense(x, weight, bias, mask, plan)


# revision 21
# speedup vs baseline: 1.0349x; 1.0349x over previous
"""Block-sparse linear kernel for Trainium2 (8 NeuronCores, Bass/Tile).

Computes out = x @ (weight*mask).T + bias for
  x [4, 2048, 4096] f32, weight [4096, 4096] f32, mask [4096,4096] bool,
  bias [4096] f32  ->  out [4, 2048, 4096] f32.

Strategy (data-parallel over tokens, 8 cores x 1024 tokens each):
  The 16x16 block mask is coarsened by greedy max-overlap matching into
  supercells of 2 input blocks (K=32) x 4 output blocks (M=64).  Only
  nonzero supercells are computed, as [32,64,512] PE-tiled fp16 matmuls
  (fp32 accumulate in PSUM) on 8 concurrent tensor-engine slots
  (4 row groups x 2 column positions).  ~3554 supercells vs 16384 dense.

  On top of the original schedule:
  - redundant same-engine dependency edges are pruned before semaphore
    assignment (engines complete in program order), removing the
    serialized per-matmul semaphore increments;
  - each set's weights go out as ONE padded [128, w*64] DMA instead of
    four ragged ones, keeping the sync queue short;
  - the 4-bank drain is split across engines with single-PSUM-read ops
    (ACT: bias add + copy, DVE: two adds, GpSimd: final add, fp16 out),
    halving VectorE load and output DMA bytes.

  Falls back to a dense fp16 kernel when the mask is not sparse enough.
"""

import sys

for _p in ("/opt/trn_rl_repo",):
    if _p not in sys.path:
        sys.path.insert(0, _p)

import numpy as np

import concourse.bacc as bacc
import concourse.mybir as mybir
import concourse.tile as tile
import concourse.tile_sem_assignment as _tsa
from concourse import bass_utils

P = 128
IN = 4096
OUT = 4096
BLK = 16
NB = IN // BLK  # 256 blocks per dim
NPAIR = NB // 2  # 128 input pairs
KO = IN // P  # 32
M = 64  # out-features per supercell
NG = OUT // M  # 64 output groups
NSET = NG // 2  # 32 sets (2 col positions)
N_CORES = 8
TOK = 1024
NCHUNK = 512
NT = TOK // NCHUNK  # 2
F16 = mybir.dt.float16
F32 = mybir.dt.float32

SPARSE_MAX_CELLS = 5400


# ------------------------------------------------------- dependency pruning
# Engines complete instructions in program order (PE matmuls are
# pc-monotone in start and end; ACT/DVE queues are strict FIFO), so a
# consumer depending on several producers from one engine only needs the
# latest edge.  GpSimd (8 Q7 cores) and DMAs (multiple queues) excluded.

_PRUNABLE = {
    ("Matmult", mybir.EngineType.PE),
    ("Activation", mybir.EngineType.Activation),
    ("Activation", mybir.EngineType.DVE),
    ("TensorTensor", mybir.EngineType.DVE),
    ("TensorScalarPtr", mybir.EngineType.DVE),
    ("TensorCopy", mybir.EngineType.DVE),
    ("Memset", mybir.EngineType.DVE),
}

_pruned_ids = set()


def _prune_blocks(blocks):
    if id(blocks) in _pruned_ids:
        return
    _pruned_ids.add(id(blocks))
    for bb_name, insts in blocks.items():
        order = {}
        by_name = {}
        for k, inst in enumerate(insts):
            order[inst.name] = k
            by_name[inst.name] = inst
        for d in insts:
            deps = d.sync_dependency_names() or ()
            if len(deps) < 2:
                continue
            groups = {}
            for pname in deps:
                p = by_name.get(pname)
                if p is None:
                    continue
                if (p.opcode, p.engine) in _PRUNABLE:
                    groups.setdefault(p.engine, []).append(p)
            for eng, plist in groups.items():
                if len(plist) < 2:
                    continue
                plist.sort(key=lambda p: order[p.name])
                for p in plist[:-1]:
                    d.remove_dependency(p.name)
                    if p.descendants is not None:
                        p.descendants.discard(d.name)


_orig_assign_ticks = _tsa.TileClockTick.assign_ticks
_install_done = False


def _install_prune():
    global _install_done
    if _install_done:
        return
    _install_done = True

    def assign_ticks_pruned(self, bb_name):
        _prune_blocks(self.ordered_instructions_by_block)
        return _orig_assign_ticks(self, bb_name)

    _tsa.TileClockTick.assign_ticks = assign_ticks_pruned


# ---------------------------------------------------------------- matching


def greedy_pair(support):
    """support: [N, D] bool rows. Pair rows maximizing overlap; [N/2, 2]."""
    N = support.shape[0]
    A = support.astype(np.int32)
    O = A @ A.T
    np.fill_diagonal(O, -1)
    pairs = []
    for _ in range(N // 2):
        idx = int(np.argmax(O))
        i, j = divmod(idx, N)
        pairs.append((i, j))
        O[i, :] = -1
        O[:, i] = -1
        O[j, :] = -1
        O[:, j] = -1
    return np.array(pairs, dtype=np.int64)


def analyze_mask(mask):
    """Returns (in_pairs [128,2], groups [64][4 block ids], sc64 [64,128])."""
    bm = mask.reshape(NB, BLK, NB, BLK).any(axis=(1, 3))  # [out_blk, in_blk]
    in_pairs = greedy_pair(bm.T)
    best = None
    for _ in range(4):
        bmc = bm[:, in_pairs[:, 0]] | bm[:, in_pairs[:, 1]]  # [256, 128]
        out_pairs = greedy_pair(bmc)
        sc32 = bmc[out_pairs[:, 0]] | bmc[out_pairs[:, 1]]
        rp = greedy_pair(sc32)
        sc64 = sc32[rp[:, 0]] | sc32[rp[:, 1]]  # [64, 128]
        groups = np.array(
            [[out_pairs[a][0], out_pairs[a][1], out_pairs[b][0], out_pairs[b][1]]
             for a, b in rp], dtype=np.int64)
        cells = int(sc64.sum())
        if best is None or cells < best[0]:
            best = (cells, in_pairs.copy(), groups, sc64)
        bg = np.zeros((NG, NB), dtype=bool)  # [group, in_blk]
        for g in range(NG):
            bg[g] = bm[groups[g]].any(axis=0)
        in_pairs = greedy_pair(bg.T)
    _, in_pairs, groups, _ = best

    # refine the block->group assignment by hill-climbing on total cells
    bp = bm[:, in_pairs[:, 0]] | bm[:, in_pairs[:, 1]]  # [out_blk, pair]
    groups = groups.copy()
    cnt = np.zeros((NG, NPAIR), dtype=np.int16)
    for g in range(NG):
        cnt[g] = bp[groups[g]].sum(axis=0)
    cells = int((cnt > 0).sum())
    rng = np.random.default_rng(1)
    gi = np.zeros(NB, dtype=np.int64)  # block -> group
    pos = np.zeros(NB, dtype=np.int64)
    for g in range(NG):
        for k in range(4):
            gi[groups[g][k]] = g
            pos[groups[g][k]] = k
    for _ in range(60000):
        u, v = rng.integers(0, NB, 2)
        g1, g2 = gi[u], gi[v]
        if g1 == g2:
            continue
        n1 = cnt[g1] - bp[u] + bp[v]
        n2 = cnt[g2] - bp[v] + bp[u]
        d = (int((n1 > 0).sum()) + int((n2 > 0).sum())
             - int((cnt[g1] > 0).sum()) - int((cnt[g2] > 0).sum()))
        if d <= 0:
            cnt[g1], cnt[g2] = n1, n2
            k1, k2 = pos[u], pos[v]
            groups[g1][k1], groups[g2][k2] = v, u
            gi[u], gi[v] = g2, g1
            pos[u], pos[v] = k2, k1
            cells += d
    sc64 = cnt > 0
    return in_pairs, groups, sc64


def _rebalance_pairs(sc64, set_G):
    """Permute pair layout positions to even out per-set slot loads."""
    rng = np.random.default_rng(0)
    perm = np.arange(NPAIR)
    use_ids = [[np.nonzero(sc64[set_G[s][c]])[0] for c in range(2)]
               for s in range(NSET)]

    def set_cost(p):
        rg = np.zeros(NPAIR, dtype=np.int64)
        rg[p] = np.arange(NPAIR) % 4
        tot = 0
        for s in range(NSET):
            mx = 1
            for c in range(2):
                ids = use_ids[s][c]
                if len(ids):
                    mx = max(mx, int(np.bincount(rg[ids], minlength=4).max()))
            tot += mx
        return tot

    best = set_cost(perm)
    for _ in range(4000):
        i, j = rng.integers(0, NPAIR, 2)
        if i == j or (i % 4) == (j % 4):
            continue
        perm[[i, j]] = perm[[j, i]]
        c = set_cost(perm)
        if c <= best:
            best = c
        else:
            perm[[i, j]] = perm[[j, i]]
    return perm


def build_schedule(sc64):
    counts = sc64.sum(axis=1)
    order = np.argsort(-counts)
    set_G = [[int(order[s * 2 + c]) for c in range(2)] for s in range(NSET)]
    perm = _rebalance_pairs(sc64, set_G)
    pair_pos = np.empty(NPAIR, dtype=np.int64)
    pair_pos[perm] = np.arange(NPAIR)
    slot_lists = []
    for s in range(NSET):
        rows = []
        for r in range(4):
            cols = []
            for c in range(2):
                G = set_G[s][c]
                cs = sorted(int(pair_pos[i]) for i in np.nonzero(sc64[G])[0]
                            if pair_pos[i] % 4 == r)
                cols.append(cs)
            rows.append(cols)
        slot_lists.append(rows)
    return set_G, slot_lists, perm


# ---------------------------------------------------------------- device


def build_sparse(slot_lists, dt=F16):
    _install_prune()
    nc = bacc.Bacc("TRN2", target_bir_lowering=False, debug=False)

    n_sr = np.zeros((NSET, 4), dtype=np.int64)
    for s in range(NSET):
        for r in range(4):
            n_sr[s, r] = sum(max(1, len(slot_lists[s][r][c])) for c in range(2))
    wsz = n_sr.max(axis=1)  # per-set uniform class width (padded)
    maxn = int(wsz.max())
    w_offs = np.zeros(NSET, dtype=np.int64)
    off = 0
    for s in range(NSET):
        w_offs[s] = off
        off += P * int(wsz[s]) * M
    w_words = int(off)

    xT = nc.dram_tensor("xT", [P, KO, TOK], dt, kind="ExternalInput")
    w = nc.dram_tensor("w", [w_words], dt, kind="ExternalInput")
    bias = nc.dram_tensor("bias", [P, NSET], F32, kind="ExternalInput")
    outT = nc.dram_tensor("outT", [NSET, P, TOK], F16, kind="ExternalOutput")

    with tile.TileContext(nc) as tc:
        with (
            tc.tile_pool(name="x_pool", bufs=1) as x_pool,
            tc.tile_pool(name="const", bufs=1) as const_pool,
            tc.tile_pool(name="w_pool", bufs=6) as w_pool,
            tc.tile_pool(name="drain_pool", bufs=4) as drain_pool,
            tc.tile_pool(name="out_pool", bufs=6) as out_pool,
            tc.tile_pool(name="psum", bufs=1, space="PSUM") as psum_pool,
        ):
            XSPLIT = 4
            NXG = KO // XSPLIT
            xts = []
            for g in range(NXG):
                xg = x_pool.tile([P, XSPLIT, TOK], dt, name=f"x{g}", tag=f"x{g}")
                xts.append(xg)

            def emit_x_dma(g):
                nc.sync.dma_start(
                    xts[g][:], xT.ap()[:, g * XSPLIT: (g + 1) * XSPLIT]
                )

            emit_x_dma(0)
            bt = const_pool.tile([P, NSET], F32, name="bt")
            nc.sync.dma_start(bt[:], bias.ap())

            def emit_w_dma(s, wt):
                nwords = P * int(wsz[s]) * M
                o = int(w_offs[s])
                src = w.ap()[o:o + nwords].rearrange("(p f) -> p f", p=P)
                nc.sync.dma_start(wt[:, :int(wsz[s]) * M], src)

            # prefetch first four sets' weights ahead of the bulk of x
            pre_wt = {}
            for s in range(4):
                wt = w_pool.tile([P, maxn * M], dt, tag="w", name="wt")
                emit_w_dma(s, wt)
                pre_wt[s] = wt
            for g in range(1, NXG):
                emit_x_dma(g)

            for s in range(NSET):
                if s in pre_wt:
                    wt = pre_wt[s]
                else:
                    wt = w_pool.tile([P, maxn * M], dt, tag="w", name="wt")
                    emit_w_dma(s, wt)

                ps = [
                    [psum_pool.tile([P, NCHUNK], F32, tag=f"ps_{n}_{r}",
                                    name=f"ps_{n}_{r}")
                     for r in range(4)]
                    for n in range(NT)
                ]

                items = {}
                for r in range(4):
                    for c in range(2):
                        lst = slot_lists[s][r][c]
                        items[(r, c)] = lst if lst else [-1]
                slot_widx = {}
                for r in range(4):
                    k = 0
                    for c in range(2):
                        slot_widx[(r, c)] = k
                        k += len(items[(r, c)])
                nsteps = max(len(items[(r, c)])
                             for r in range(4) for c in range(2))

                for n in range(NT):
                    for st in range(nsteps):
                        for r in range(4):
                            for c in range(2):
                                lst = items[(r, c)]
                                if st >= len(lst):
                                    continue
                                ci = lst[st]
                                wi = slot_widx[(r, c)] + st
                                lhsT = wt[32 * r: 32 * r + 32,
                                          wi * M: (wi + 1) * M]
                                start = st == 0
                                stop = st == len(lst) - 1
                                if ci < 0:
                                    ko, rg = 0, r
                                else:
                                    ko, rg = ci // 4, ci % 4
                                rhs = xts[ko // XSPLIT][
                                    32 * rg: 32 * rg + 32, ko % XSPLIT,
                                    n * NCHUNK: (n + 1) * NCHUNK]
                                nc.tensor.matmul(
                                    ps[n][r][M * c: M * c + M, :], lhsT, rhs,
                                    start=start, stop=stop,
                                    tile_position=(32 * r, M * c),
                                )
                    at = drain_pool.tile([P, NCHUNK], F32, tag="a", name="at")
                    nc.scalar.activation(
                        at[:], ps[n][0][:],
                        mybir.ActivationFunctionType.Identity,
                        bias=bt[:, s: s + 1],
                    )
                    bt2 = drain_pool.tile([P, NCHUNK], F32, tag="b",
                                          name="bt2")
                    nc.scalar.copy(bt2[:], ps[n][3][:])
                    ut = drain_pool.tile([P, NCHUNK], F32, tag="u", name="ut")
                    nc.vector.tensor_tensor(ut[:], ps[n][1][:], at[:],
                                            mybir.AluOpType.add)
                    nc.vector.tensor_tensor(ut[:], ut[:], ps[n][2][:],
                                            mybir.AluOpType.add)
                    ot = out_pool.tile([P, NCHUNK], F16, tag="ot", name="ot")
                    nc.gpsimd.tensor_tensor(ot[:], bt2[:], ut[:],
                                            mybir.AluOpType.add)
                    nc.sync.dma_start(
                        outT.ap()[s, :, n * NCHUNK: (n + 1) * NCHUNK], ot[:]
                    )
    nc.compile()
    return nc, n_sr


def build_dense(dt=F16):
    """Dense fallback: [128,128,512] matmuls, K-contiguous, out-group major."""
    NM = OUT // P
    nc = bacc.Bacc("TRN2", target_bir_lowering=False, debug=False)
    xT = nc.dram_tensor("xT", [P, KO, TOK], dt, kind="ExternalInput")
    w = nc.dram_tensor("w", [NM, P, KO, P], dt, kind="ExternalInput")
    bias = nc.dram_tensor("bias", [P, NM], F32, kind="ExternalInput")
    outT = nc.dram_tensor("outT", [NM, P, TOK], F32, kind="ExternalOutput")

    with tile.TileContext(nc) as tc:
        with (
            tc.tile_pool(name="x_pool", bufs=1) as x_pool,
            tc.tile_pool(name="const", bufs=1) as const_pool,
            tc.tile_pool(name="w_pool", bufs=3) as w_pool,
            tc.tile_pool(name="out_pool", bufs=4) as out_pool,
            tc.tile_pool(name="psum", bufs=2, space="PSUM") as psum_pool,
        ):
            xt = x_pool.tile([P, KO, TOK], dt, name="xt")
            nc.sync.dma_start(xt[:], xT.ap())
            bt = const_pool.tile([P, NM], F32, name="bt")
            nc.sync.dma_start(bt[:], bias.ap())
            for m in range(NM):
                wt = w_pool.tile([P, KO, P], dt, name="wt")
                nc.sync.dma_start(wt[:], w.ap()[m])
                for n in range(NT):
                    psd = psum_pool.tile([P, NCHUNK], F32, name="psd")
                    for ko in range(KO):
                        nc.tensor.matmul(
                            psd[:], wt[:, ko],
                            xt[:, ko, n * NCHUNK: (n + 1) * NCHUNK],
                            start=(ko == 0), stop=(ko == KO - 1),
                        )
                    ot = out_pool.tile([P, NCHUNK], F32, name="ot")
                    nc.scalar.activation(
                        ot[:], psd[:], mybir.ActivationFunctionType.Identity,
                        bias=bt[:, m: m + 1],
                    )
                    nc.sync.dma_start(
                        outT.ap()[m, :, n * NCHUNK: (n + 1) * NCHUNK], ot[:]
                    )
    nc.compile()
    return nc


# ---------------------------------------------------------------- packing


def group_feats(groups, G):
    return np.concatenate([np.arange(b * BLK, (b + 1) * BLK)
                           for b in groups[G]])


def pack_weights(weight, mask, in_pairs, groups, set_G, slot_lists, n_sr):
    wm = weight.astype(np.float32) * mask
    wsz = n_sr.max(axis=1)
    total = int(P * (wsz * M).sum())
    out = np.zeros(total, dtype=np.float32)
    off = 0
    for s in range(NSET):
        nn = int(wsz[s])
        blockbuf = np.zeros((P, nn * M), dtype=np.float32)
        for r in range(4):
            k = 0
            for c in range(2):
                G = set_G[s][c]
                ofeat = group_feats(groups, G)
                lst = slot_lists[s][r][c]
                if not lst:
                    k += 1
                    continue
                for ci in lst:
                    a, b = in_pairs[ci]
                    ifeat = np.concatenate(
                        [np.arange(a * BLK, (a + 1) * BLK),
                         np.arange(b * BLK, (b + 1) * BLK)]
                    )
                    blockbuf[32 * r:32 * r + 32, k * M:(k + 1) * M] = \
                        wm[np.ix_(ofeat, ifeat)].T
                    k += 1
        nw = P * nn * M
        out[off:off + nw] = blockbuf.reshape(-1)
        off += nw
    return out.astype(np.float16)


def pack_x_shard(x_shard, in_pairs):
    src_feat = np.empty((P, KO), dtype=np.int64)
    for i in range(NPAIR):
        a, b = in_pairs[i]
        ko, rg = i // 4, i % 4
        src_feat[rg * 32: rg * 32 + 16, ko] = np.arange(a * BLK, (a + 1) * BLK)
        src_feat[rg * 32 + 16: rg * 32 + 32, ko] = np.arange(b * BLK,
                                                             (b + 1) * BLK)
    xs = x_shard.astype(np.float16)
    xt = xs.T[src_feat.reshape(-1)].reshape(P, KO, TOK)
    return np.ascontiguousarray(xt)


def pack_bias(bias, groups, set_G):
    bp = np.zeros((P, NSET), dtype=np.float32)
    b = bias.astype(np.float32)
    for s in range(NSET):
        for c in range(2):
            bp[M * c: M * c + M, s] = b[group_feats(groups, set_G[s][c])]
    return bp


def out_feat_map(groups, set_G):
    m = np.empty(OUT, dtype=np.int64)
    for s in range(NSET):
        for c in range(2):
            m[s * P + M * c: s * P + M * c + M] = group_feats(
                groups, set_G[s][c])
    return m


# ---------------------------------------------------------------- entry

_CACHE = {}


def _run_sparse(x, weight, bias, mask, plan):
    nc, in_pairs, groups, set_G, slot_lists, n_sr = plan
    w_flat = pack_weights(weight, mask, in_pairs, groups, set_G,
                          slot_lists, n_sr)
    bias_p = pack_bias(bias, groups, set_G)
    B, S = x.shape[0], x.shape[1]
    xf = np.ascontiguousarray(x.reshape(B * S, IN))
    in_maps = []
    for cidx in range(N_CORES):
        xs = xf[cidx * TOK: (cidx + 1) * TOK]
        in_maps.append({"xT": pack_x_shard(xs, in_pairs), "w": w_flat,
                        "bias": bias_p})
    res = bass_utils.run_bass_kernel_spmd(
        nc, in_maps, core_ids=list(range(N_CORES)))
    fmap = out_feat_map(groups, set_G)
    outs = []
    for cidx in range(N_CORES):
        o = res.results[cidx]["outT"].reshape(OUT, TOK).astype(np.float32)
        unperm = np.empty_like(o)
        unperm[fmap] = o
        outs.append(unperm.T)
    full = np.concatenate(outs, axis=0)
    return np.ascontiguousarray(full.reshape(B, S, OUT))


def _run_dense(x, weight, bias, mask, nc):
    NM = OUT // P
    wm = (weight.astype(np.float32) * mask).astype(np.float16)
    w_packed = np.ascontiguousarray(
        wm.T.reshape(KO, P, NM, P).transpose(2, 1, 0, 3))
    bias_p = np.ascontiguousarray(bias.astype(np.float32).reshape(NM, P).T)
    B, S = x.shape[0], x.shape[1]
    xf = np.ascontiguousarray(x.reshape(B * S, IN))
    in_maps = []
    for cidx in range(N_CORES):
        xs = xf[cidx * TOK: (cidx + 1) * TOK].astype(np.float16)
        xp = np.ascontiguousarray(xs.T.reshape(KO, P, TOK).transpose(1, 0, 2))
        in_maps.append({"xT": xp, "w": w_packed, "bias": bias_p})
    res = bass_utils.run_bass_kernel_spmd(
        nc, in_maps, core_ids=list(range(N_CORES)))
    outs = []
    for cidx in range(N_CORES):
        o = res.results[cidx]["outT"].reshape(OUT, TOK)
        outs.append(o.T)
    full = np.concatenate(outs, axis=0)
    return np.ascontiguousarray(full.reshape(B, S, OUT).astype(np.float32))


def kernel(x, weight, bias, mask):
    x = np.asarray(x, dtype=np.float32)
    weight = np.asarray(weight, dtype=np.float32)
    bias = np.asarray(bias, dtype=np.float32)
    mask = np.asarray(mask).astype(bool)
    assert x.shape == (4, 2048, IN) and weight.shape == (OUT, IN)

    key = hash(mask.tobytes())
    if key not in _CACHE:
        in_pairs, groups, sc64 = analyze_mask(mask)
        cells = int(sc64.sum())
        if cells <= SPARSE_MAX_CELLS:
            set_G, slot_lists, perm = build_schedule(sc64)
            in_pairs = in_pairs[perm]
            nc, n_sr = build_sparse(slot_lists)
            _CACHE[key] = ("sparse",
                           (nc, in_pairs, groups, set_G, slot_lists, n_sr))
        else:
            _CACHE[key] = ("dense", build_dense())
    kind, plan = _CACHE[key]
    if kind == "sparse":
        return _run_sparse(x, weight, bias, mask, plan)
    return _run_dense(x, weight, bias, mask, plan)


# revision 23
# speedup vs baseline: 1.1124x; 1.0749x over previous
"""Block-sparse linear kernel for Trainium2 (8 NeuronCores, Bass/Tile).

Computes out = x @ (weight*mask).T + bias for
  x [4, 2048, 4096] f32, weight [4096, 4096] f32, mask [4096,4096] bool,
  bias [4096] f32  ->  out [4, 2048, 4096] f32.

Strategy (data-parallel over tokens, 8 cores x 1024 tokens each):
  The 16x16 block mask is coarsened by greedy max-overlap matching into
  supercells of 2 input blocks (K=32) x 4 output blocks (M=64).  Only
  nonzero supercells are computed, as [32,64,512] PE-tiled fp16 matmuls
  (fp32 accumulate in PSUM) on 8 concurrent tensor-engine slots
  (4 row groups x 2 column positions).  ~3554 supercells vs 16384 dense.

  On top of the original schedule:
  - redundant same-engine dependency edges are pruned before semaphore
    assignment (engines complete in program order), removing the
    serialized per-matmul semaphore increments;
  - each set's weights go out as ONE padded [128, w*64] DMA instead of
    four ragged ones, keeping the sync queue short;
  - the 4-bank drain is split across engines with single-PSUM-read ops
    (ACT: bias add + copy, DVE: two adds, GpSimd: final add, fp16 out),
    halving VectorE load and output DMA bytes.

  Falls back to a dense fp16 kernel when the mask is not sparse enough.
"""

import sys

for _p in ("/opt/trn_rl_repo",):
    if _p not in sys.path:
        sys.path.insert(0, _p)

import numpy as np

import concourse.bacc as bacc
import concourse.mybir as mybir
import concourse.tile as tile
import concourse.tile_sem_assignment as _tsa
from concourse import bass_utils

P = 128
IN = 4096
OUT = 4096
BLK = 16
NB = IN // BLK  # 256 blocks per dim
NPAIR = NB // 2  # 128 input pairs
KO = IN // P  # 32
M = 64  # out-features per supercell
NG = OUT // M  # 64 output groups
NSET = NG // 2  # 32 sets (2 col positions)
N_CORES = 8
TOK = 1024
NCHUNK = 512
NT = TOK // NCHUNK  # 2
F16 = mybir.dt.float16
F32 = mybir.dt.float32

SPARSE_MAX_CELLS = 5400


# ------------------------------------------------------- dependency pruning
# Engines complete instructions in program order (PE matmuls are
# pc-monotone in start and end; ACT/DVE queues are strict FIFO), so a
# consumer depending on several producers from one engine only needs the
# latest edge.  GpSimd (8 Q7 cores) and DMAs (multiple queues) excluded.

_PRUNABLE = {
    ("Matmult", mybir.EngineType.PE),
    ("Activation", mybir.EngineType.Activation),
    ("Activation", mybir.EngineType.DVE),
    ("TensorTensor", mybir.EngineType.DVE),
    ("TensorScalarPtr", mybir.EngineType.DVE),
    ("TensorCopy", mybir.EngineType.DVE),
    ("Memset", mybir.EngineType.DVE),
}

_pruned_ids = set()


def _prune_blocks(blocks):
    if id(blocks) in _pruned_ids:
        return
    _pruned_ids.add(id(blocks))
    for bb_name, insts in blocks.items():
        order = {}
        by_name = {}
        for k, inst in enumerate(insts):
            order[inst.name] = k
            by_name[inst.name] = inst
        for d in insts:
            deps = d.sync_dependency_names() or ()
            if len(deps) < 2:
                continue
            groups = {}
            for pname in deps:
                p = by_name.get(pname)
                if p is None:
                    continue
                if (p.opcode, p.engine) in _PRUNABLE:
                    groups.setdefault(p.engine, []).append(p)
            for eng, plist in groups.items():
                if len(plist) < 2:
                    continue
                plist.sort(key=lambda p: order[p.name])
                for p in plist[:-1]:
                    d.remove_dependency(p.name)
                    if p.descendants is not None:
                        p.descendants.discard(d.name)


_orig_assign_ticks = _tsa.TileClockTick.assign_ticks
_install_done = False


def _install_prune():
    global _install_done
    if _install_done:
        return
    _install_done = True

    def assign_ticks_pruned(self, bb_name):
        _prune_blocks(self.ordered_instructions_by_block)
        return _orig_assign_ticks(self, bb_name)

    _tsa.TileClockTick.assign_ticks = assign_ticks_pruned


# ---------------------------------------------------------------- matching


def greedy_pair(support):
    """support: [N, D] bool rows. Pair rows maximizing overlap; [N/2, 2]."""
    N = support.shape[0]
    A = support.astype(np.int32)
    O = A @ A.T
    np.fill_diagonal(O, -1)
    pairs = []
    for _ in range(N // 2):
        idx = int(np.argmax(O))
        i, j = divmod(idx, N)
        pairs.append((i, j))
        O[i, :] = -1
        O[:, i] = -1
        O[j, :] = -1
        O[:, j] = -1
    return np.array(pairs, dtype=np.int64)


def analyze_mask(mask):
    """Returns (in_pairs [128,2], groups [64][4 block ids], sc64 [64,128])."""
    bm = mask.reshape(NB, BLK, NB, BLK).any(axis=(1, 3))  # [out_blk, in_blk]
    in_pairs = greedy_pair(bm.T)
    best = None
    for _ in range(4):
        bmc = bm[:, in_pairs[:, 0]] | bm[:, in_pairs[:, 1]]  # [256, 128]
        out_pairs = greedy_pair(bmc)
        sc32 = bmc[out_pairs[:, 0]] | bmc[out_pairs[:, 1]]
        rp = greedy_pair(sc32)
        sc64 = sc32[rp[:, 0]] | sc32[rp[:, 1]]  # [64, 128]
        groups = np.array(
            [[out_pairs[a][0], out_pairs[a][1], out_pairs[b][0], out_pairs[b][1]]
             for a, b in rp], dtype=np.int64)
        cells = int(sc64.sum())
        if best is None or cells < best[0]:
            best = (cells, in_pairs.copy(), groups, sc64)
        bg = np.zeros((NG, NB), dtype=bool)  # [group, in_blk]
        for g in range(NG):
            bg[g] = bm[groups[g]].any(axis=0)
        in_pairs = greedy_pair(bg.T)
    _, in_pairs, groups, _ = best

    # refine the block->group assignment by hill-climbing on total cells
    bp = bm[:, in_pairs[:, 0]] | bm[:, in_pairs[:, 1]]  # [out_blk, pair]
    groups = groups.copy()
    cnt = np.zeros((NG, NPAIR), dtype=np.int16)
    for g in range(NG):
        cnt[g] = bp[groups[g]].sum(axis=0)
    cells = int((cnt > 0).sum())
    rng = np.random.default_rng(1)
    gi = np.zeros(NB, dtype=np.int64)  # block -> group
    pos = np.zeros(NB, dtype=np.int64)
    for g in range(NG):
        for k in range(4):
            gi[groups[g][k]] = g
            pos[groups[g][k]] = k
    for _ in range(60000):
        u, v = rng.integers(0, NB, 2)
        g1, g2 = gi[u], gi[v]
        if g1 == g2:
            continue
        n1 = cnt[g1] - bp[u] + bp[v]
        n2 = cnt[g2] - bp[v] + bp[u]
        d = (int((n1 > 0).sum()) + int((n2 > 0).sum())
             - int((cnt[g1] > 0).sum()) - int((cnt[g2] > 0).sum()))
        if d <= 0:
            cnt[g1], cnt[g2] = n1, n2
            k1, k2 = pos[u], pos[v]
            groups[g1][k1], groups[g2][k2] = v, u
            gi[u], gi[v] = g2, g1
            pos[u], pos[v] = k2, k1
            cells += d
    sc64 = cnt > 0
    return in_pairs, groups, sc64


def _rebalance_pairs(sc64, set_G):
    """Permute pair layout positions to even out per-set slot loads."""
    rng = np.random.default_rng(0)
    perm = np.arange(NPAIR)
    use_ids = [[np.nonzero(sc64[set_G[s][c]])[0] for c in range(2)]
               for s in range(NSET)]

    def set_cost(p):
        rg = np.zeros(NPAIR, dtype=np.int64)
        rg[p] = np.arange(NPAIR) % 4
        tot = 0
        for s in range(NSET):
            mx = 1
            for c in range(2):
                ids = use_ids[s][c]
                if len(ids):
                    mx = max(mx, int(np.bincount(rg[ids], minlength=4).max()))
            tot += mx
        return tot

    best = set_cost(perm)
    for _ in range(4000):
        i, j = rng.integers(0, NPAIR, 2)
        if i == j or (i % 4) == (j % 4):
            continue
        perm[[i, j]] = perm[[j, i]]
        c = set_cost(perm)
        if c <= best:
            best = c
        else:
            perm[[i, j]] = perm[[j, i]]
    return perm


def build_schedule(sc64):
    counts = sc64.sum(axis=1)
    order = np.argsort(-counts)
    set_G = [[int(order[s * 2 + c]) for c in range(2)] for s in range(NSET)]
    perm = _rebalance_pairs(sc64, set_G)
    pair_pos = np.empty(NPAIR, dtype=np.int64)
    pair_pos[perm] = np.arange(NPAIR)
    slot_lists = []
    for s in range(NSET):
        rows = []
        for r in range(4):
            cols = []
            for c in range(2):
                G = set_G[s][c]
                cs = sorted(int(pair_pos[i]) for i in np.nonzero(sc64[G])[0]
                            if pair_pos[i] % 4 == r)
                cols.append(cs)
            rows.append(cols)
        slot_lists.append(rows)
    return set_G, slot_lists, perm


# ---------------------------------------------------------------- device


def build_sparse(slot_lists, dt=F16):
    _install_prune()
    nc = bacc.Bacc("TRN2", target_bir_lowering=False, debug=False)

    n_sr = np.zeros((NSET, 4), dtype=np.int64)
    for s in range(NSET):
        for r in range(4):
            n_sr[s, r] = sum(max(1, len(slot_lists[s][r][c])) for c in range(2))
    wsz = n_sr.max(axis=1)  # per-set uniform class width (padded)
    maxn = int(wsz.max())
    w_offs = np.zeros(NSET, dtype=np.int64)
    off = 0
    for s in range(NSET):
        w_offs[s] = off
        off += P * int(wsz[s]) * M
    w_words = int(off)

    xT = nc.dram_tensor("xT", [P, KO, TOK], dt, kind="ExternalInput")
    w = nc.dram_tensor("w", [w_words], dt, kind="ExternalInput")
    bias = nc.dram_tensor("bias", [P, NSET], F32, kind="ExternalInput")
    outT = nc.dram_tensor("outT", [NSET, P, TOK], F16, kind="ExternalOutput")

    with tile.TileContext(nc) as tc:
        with (
            tc.tile_pool(name="x_pool", bufs=1) as x_pool,
            tc.tile_pool(name="const", bufs=1) as const_pool,
            tc.tile_pool(name="w_pool", bufs=6) as w_pool,
            tc.tile_pool(name="drain_pool", bufs=4) as drain_pool,
            tc.tile_pool(name="out_pool", bufs=6) as out_pool,
            tc.tile_pool(name="psum", bufs=1, space="PSUM") as psum_pool,
        ):
            XSPLIT = 4
            NXG = KO // XSPLIT
            xts = []
            for g in range(NXG):
                xg = x_pool.tile([P, XSPLIT, TOK], dt, name=f"x{g}", tag=f"x{g}")
                xts.append(xg)

            def emit_x_dma(g):
                nc.sync.dma_start(
                    xts[g][:], xT.ap()[:, g * XSPLIT: (g + 1) * XSPLIT]
                )

            emit_x_dma(0)
            bt = const_pool.tile([P, NSET], F32, name="bt")
            nc.sync.dma_start(bt[:], bias.ap())

            def emit_w_dma(s, wt):
                nwords = P * int(wsz[s]) * M
                o = int(w_offs[s])
                src = w.ap()[o:o + nwords].rearrange("(p f) -> p f", p=P)
                nc.sync.dma_start(wt[:, :int(wsz[s]) * M], src)

            # prefetch first four sets' weights ahead of the bulk of x
            pre_wt = {}
            for s in range(4):
                wt = w_pool.tile([P, maxn * M], dt, tag="w", name="wt")
                emit_w_dma(s, wt)
                pre_wt[s] = wt
            for g in range(1, NXG):
                emit_x_dma(g)

            for s in range(NSET):
                wt = pre_wt.pop(s)
                for sp in (s + 3, s + 4):
                    if sp < NSET and sp not in pre_wt and sp > s:
                        wtp = w_pool.tile([P, maxn * M], dt, tag="w",
                                          name="wt")
                        emit_w_dma(sp, wtp)
                        pre_wt[sp] = wtp

                ps = [
                    [psum_pool.tile([P, NCHUNK], F32, tag=f"ps_{n}_{r}",
                                    name=f"ps_{n}_{r}")
                     for r in range(4)]
                    for n in range(NT)
                ]

                items = {}
                for r in range(4):
                    for c in range(2):
                        lst = slot_lists[s][r][c]
                        items[(r, c)] = lst if lst else [-1]
                slot_widx = {}
                for r in range(4):
                    k = 0
                    for c in range(2):
                        slot_widx[(r, c)] = k
                        k += len(items[(r, c)])
                nsteps = max(len(items[(r, c)])
                             for r in range(4) for c in range(2))

                for n in range(NT):
                    for st in range(nsteps):
                        for r in range(4):
                            for c in range(2):
                                lst = items[(r, c)]
                                if st >= len(lst):
                                    continue
                                ci = lst[st]
                                wi = slot_widx[(r, c)] + st
                                lhsT = wt[32 * r: 32 * r + 32,
                                          wi * M: (wi + 1) * M]
                                start = st == 0
                                stop = st == len(lst) - 1
                                if ci < 0:
                                    ko, rg = 0, r
                                else:
                                    ko, rg = ci // 4, ci % 4
                                rhs = xts[ko // XSPLIT][
                                    32 * rg: 32 * rg + 32, ko % XSPLIT,
                                    n * NCHUNK: (n + 1) * NCHUNK]
                                nc.tensor.matmul(
                                    ps[n][r][M * c: M * c + M, :], lhsT, rhs,
                                    start=start, stop=stop,
                                    tile_position=(32 * r, M * c),
                                )
                    at = drain_pool.tile([P, NCHUNK], F32, tag="a", name="at")
                    nc.scalar.activation(
                        at[:], ps[n][0][:],
                        mybir.ActivationFunctionType.Identity,
                        bias=bt[:, s: s + 1],
                    )
                    bt2 = drain_pool.tile([P, NCHUNK], F32, tag="b",
                                          name="bt2")
                    nc.scalar.copy(bt2[:], ps[n][3][:])
                    ut = drain_pool.tile([P, NCHUNK], F32, tag="u", name="ut")
                    nc.vector.tensor_tensor(ut[:], ps[n][1][:], at[:],
                                            mybir.AluOpType.add)
                    nc.vector.tensor_tensor(ut[:], ut[:], ps[n][2][:],
                                            mybir.AluOpType.add)
                    ot = out_pool.tile([P, NCHUNK], F16, tag="ot", name="ot")
                    nc.gpsimd.tensor_tensor(ot[:], bt2[:], ut[:],
                                            mybir.AluOpType.add)
                    nc.gpsimd.dma_start(
                        outT.ap()[s, :, n * NCHUNK: (n + 1) * NCHUNK], ot[:]
                    )
    nc.compile()
    return nc, n_sr


def build_dense(dt=F16):
    """Dense fallback: [128,128,512] matmuls, K-contiguous, out-group major."""
    NM = OUT // P
    nc = bacc.Bacc("TRN2", target_bir_lowering=False, debug=False)
    xT = nc.dram_tensor("xT", [P, KO, TOK], dt, kind="ExternalInput")
    w = nc.dram_tensor("w", [NM, P, KO, P], dt, kind="ExternalInput")
    bias = nc.dram_tensor("bias", [P, NM], F32, kind="ExternalInput")
    outT = nc.dram_tensor("outT", [NM, P, TOK], F32, kind="ExternalOutput")

    with tile.TileContext(nc) as tc:
        with (
            tc.tile_pool(name="x_pool", bufs=1) as x_pool,
            tc.tile_pool(name="const", bufs=1) as const_pool,
            tc.tile_pool(name="w_pool", bufs=3) as w_pool,
            tc.tile_pool(name="out_pool", bufs=4) as out_pool,
            tc.tile_pool(name="psum", bufs=2, space="PSUM") as psum_pool,
        ):
            xt = x_pool.tile([P, KO, TOK], dt, name="xt")
            nc.sync.dma_start(xt[:], xT.ap())
            bt = const_pool.tile([P, NM], F32, name="bt")
            nc.sync.dma_start(bt[:], bias.ap())
            for m in range(NM):
                wt = w_pool.tile([P, KO, P], dt, name="wt")
                nc.sync.dma_start(wt[:], w.ap()[m])
                for n in range(NT):
                    psd = psum_pool.tile([P, NCHUNK], F32, name="psd")
                    for ko in range(KO):
                        nc.tensor.matmul(
                            psd[:], wt[:, ko],
                            xt[:, ko, n * NCHUNK: (n + 1) * NCHUNK],
                            start=(ko == 0), stop=(ko == KO - 1),
                        )
                    ot = out_pool.tile([P, NCHUNK], F32, name="ot")
                    nc.scalar.activation(
                        ot[:], psd[:], mybir.ActivationFunctionType.Identity,
                        bias=bt[:, m: m + 1],
                    )
                    nc.sync.dma_start(
                        outT.ap()[m, :, n * NCHUNK: (n + 1) * NCHUNK], ot[:]
                    )
    nc.compile()
    return nc


# ---------------------------------------------------------------- packing


def group_feats(groups, G):
    return np.concatenate([np.arange(b * BLK, (b + 1) * BLK)
                           for b in groups[G]])


def pack_weights(weight, mask, in_pairs, groups, set_G, slot_lists, n_sr):
    wm = weight.astype(np.float32) * mask
    wsz = n_sr.max(axis=1)
    total = int(P * (wsz * M).sum())
    out = np.zeros(total, dtype=np.float32)
    off = 0
    for s in range(NSET):
        nn = int(wsz[s])
        blockbuf = np.zeros((P, nn * M), dtype=np.float32)
        for r in range(4):
            k = 0
            for c in range(2):
                G = set_G[s][c]
                ofeat = group_feats(groups, G)
                lst = slot_lists[s][r][c]
                if not lst:
                    k += 1
                    continue
                for ci in lst:
                    a, b = in_pairs[ci]
                    ifeat = np.concatenate(
                        [np.arange(a * BLK, (a + 1) * BLK),
                         np.arange(b * BLK, (b + 1) * BLK)]
                    )
                    blockbuf[32 * r:32 * r + 32, k * M:(k + 1) * M] = \
                        wm[np.ix_(ofeat, ifeat)].T
                    k += 1
        nw = P * nn * M
        out[off:off + nw] = blockbuf.reshape(-1)
        off += nw
    return out.astype(np.float16)


def pack_x_shard(x_shard, in_pairs):
    src_feat = np.empty((P, KO), dtype=np.int64)
    for i in range(NPAIR):
        a, b = in_pairs[i]
        ko, rg = i // 4, i % 4
        src_feat[rg * 32: rg * 32 + 16, ko] = np.arange(a * BLK, (a + 1) * BLK)
        src_feat[rg * 32 + 16: rg * 32 + 32, ko] = np.arange(b * BLK,
                                                             (b + 1) * BLK)
    xs = x_shard.astype(np.float16)
    xt = xs.T[src_feat.reshape(-1)].reshape(P, KO, TOK)
    return np.ascontiguousarray(xt)


def pack_bias(bias, groups, set_G):
    bp = np.zeros((P, NSET), dtype=np.float32)
    b = bias.astype(np.float32)
    for s in range(NSET):
        for c in range(2):
            bp[M * c: M * c + M, s] = b[group_feats(groups, set_G[s][c])]
    return bp


def out_feat_map(groups, set_G):
    m = np.empty(OUT, dtype=np.int64)
    for s in range(NSET):
        for c in range(2):
            m[s * P + M * c: s * P + M * c + M] = group_feats(
                groups, set_G[s][c])
    return m


# ---------------------------------------------------------------- entry

_CACHE = {}


def _run_sparse(x, weight, bias, mask, plan):
    nc, in_pairs, groups, set_G, slot_lists, n_sr = plan
    w_flat = pack_weights(weight, mask, in_pairs, groups, set_G,
                          slot_lists, n_sr)
    bias_p = pack_bias(bias, groups, set_G)
    B, S = x.shape[0], x.shape[1]
    xf = np.ascontiguousarray(x.reshape(B * S, IN))
    in_maps = []
    for cidx in range(N_CORES):
        xs = xf[cidx * TOK: (cidx + 1) * TOK]
        in_maps.append({"xT": pack_x_shard(xs, in_pairs), "w": w_flat,
                        "bias": bias_p})
    res = bass_utils.run_bass_kernel_spmd(
        nc, in_maps, core_ids=list(range(N_CORES)))
    fmap = out_feat_map(groups, set_G)
    outs = []
    for cidx in range(N_CORES):
        o = res.results[cidx]["outT"].reshape(OUT, TOK).astype(np.float32)
        unperm = np.empty_like(o)
        unperm[fmap] = o
        outs.append(unperm.T)
    full = np.concatenate(outs, axis=0)
    return np.ascontiguousarray(full.reshape(B, S, OUT))


def _run_dense(x, weight, bias, mask, nc):
    NM = OUT // P
    wm = (weight.astype(np.float32) * mask).astype(np.float16)
    w_packed = np.ascontiguousarray(
        wm.T.reshape(KO, P, NM, P).transpose(2, 1, 0, 3))
    bias_p = np.ascontiguousarray(bias.astype(np.float32).reshape(NM, P).T)
    B, S = x.shape[0], x.shape[1]
    xf = np.ascontiguousarray(x.reshape(B * S, IN))
    in_maps = []
    for cidx in range(N_CORES):
        xs = xf[cidx * TOK: (cidx + 1) * TOK].astype(np.float16)
        xp = np.ascontiguousarray(xs.T.reshape(KO, P, TOK).transpose(1, 0, 2))
        in_maps.append({"xT": xp, "w": w_packed, "bias": bias_p})
    res = bass_utils.run_bass_kernel_spmd(
        nc, in_maps, core_ids=list(range(N_CORES)))
    outs = []
    for cidx in range(N_CORES):
        o = res.results[cidx]["outT"].reshape(OUT, TOK)
        outs.append(o.T)
    full = np.concatenate(outs, axis=0)
    return np.ascontiguousarray(full.reshape(B, S, OUT).astype(np.float32))


def kernel(x, weight, bias, mask):
    x = np.asarray(x, dtype=np.float32)
    weight = np.asarray(weight, dtype=np.float32)
    bias = np.asarray(bias, dtype=np.float32)
    mask = np.asarray(mask).astype(bool)
    assert x.shape == (4, 2048, IN) and weight.shape == (OUT, IN)

    key = hash(mask.tobytes())
    if key not in _CACHE:
        in_pairs, groups, sc64 = analyze_mask(mask)
        cells = int(sc64.sum())
        if cells <= SPARSE_MAX_CELLS:
            set_G, slot_lists, perm = build_schedule(sc64)
            in_pairs = in_pairs[perm]
            nc, n_sr = build_sparse(slot_lists)
            _CACHE[key] = ("sparse",
                           (nc, in_pairs, groups, set_G, slot_lists, n_sr))
        else:
            _CACHE[key] = ("dense", build_dense())
    kind, plan = _CACHE[key]
    if kind == "sparse":
        return _run_sparse(x, weight, bias, mask, plan)
    return _run_dense(x, weight, bias, mask, plan)


# revision 26
# speedup vs baseline: 1.1650x; 1.0473x over previous
"""Block-sparse linear kernel for Trainium2 (8 NeuronCores, Bass/Tile).

Computes out = x @ (weight*mask).T + bias for
  x [4, 2048, 4096] f32, weight [4096, 4096] f32, mask [4096,4096] bool,
  bias [4096] f32  ->  out [4, 2048, 4096] f32.

Strategy (data-parallel over tokens, 8 cores x 1024 tokens each):
  The 16x16 block mask is coarsened by greedy max-overlap matching into
  supercells of 2 input blocks (K=32) x 4 output blocks (M=64).  Only
  nonzero supercells are computed, as [32,64,512] PE-tiled fp16 matmuls
  (fp32 accumulate in PSUM) on 8 concurrent tensor-engine slots
  (4 row groups x 2 column positions).  ~3554 supercells vs 16384 dense.

  On top of the original schedule:
  - redundant same-engine dependency edges are pruned before semaphore
    assignment (engines complete in program order), removing the
    serialized per-matmul semaphore increments;
  - each set's weights go out as ONE padded [128, w*64] DMA instead of
    four ragged ones, keeping the sync queue short;
  - the 4-bank drain is split across engines with single-PSUM-read ops
    (ACT: bias add + copy, DVE: two adds, GpSimd: final add, fp16 out),
    halving VectorE load and output DMA bytes.

  Falls back to a dense fp16 kernel when the mask is not sparse enough.
"""

import sys

for _p in ("/opt/trn_rl_repo",):
    if _p not in sys.path:
        sys.path.insert(0, _p)

import numpy as np

import concourse.bacc as bacc
import concourse.mybir as mybir
import concourse.tile as tile
import concourse.tile_sem_assignment as _tsa
from concourse import bass_utils

P = 128
IN = 4096
OUT = 4096
BLK = 16
NB = IN // BLK  # 256 blocks per dim
NPAIR = NB // 2  # 128 input pairs
KO = IN // P  # 32
M = 64  # out-features per supercell
NG = OUT // M  # 64 output groups
NSET = NG // 2  # 32 sets (2 col positions)
N_CORES = 8
TOK = 1024
NCHUNK = 512
NT = TOK // NCHUNK  # 2
F16 = mybir.dt.float16
F32 = mybir.dt.float32

SPARSE_MAX_CELLS = 5400


# ------------------------------------------------------- dependency pruning
# Engines complete instructions in program order (PE matmuls are
# pc-monotone in start and end; ACT/DVE queues are strict FIFO), so a
# consumer depending on several producers from one engine only needs the
# latest edge.  GpSimd (8 Q7 cores) and DMAs (multiple queues) excluded.

_PRUNABLE = {
    ("Matmult", mybir.EngineType.PE),
    ("Activation", mybir.EngineType.Activation),
    ("Activation", mybir.EngineType.DVE),
    ("TensorTensor", mybir.EngineType.DVE),
    ("TensorScalarPtr", mybir.EngineType.DVE),
    ("TensorCopy", mybir.EngineType.DVE),
    ("Memset", mybir.EngineType.DVE),
}

_pruned_ids = set()


def _prune_blocks(blocks):
    if id(blocks) in _pruned_ids:
        return
    _pruned_ids.add(id(blocks))
    for bb_name, insts in blocks.items():
        order = {}
        by_name = {}
        for k, inst in enumerate(insts):
            order[inst.name] = k
            by_name[inst.name] = inst
        for d in insts:
            deps = d.sync_dependency_names() or ()
            if len(deps) < 2:
                continue
            groups = {}
            for pname in deps:
                p = by_name.get(pname)
                if p is None:
                    continue
                if (p.opcode, p.engine) in _PRUNABLE:
                    groups.setdefault(p.engine, []).append(p)
            for eng, plist in groups.items():
                if len(plist) < 2:
                    continue
                plist.sort(key=lambda p: order[p.name])
                for p in plist[:-1]:
                    d.remove_dependency(p.name)
                    if p.descendants is not None:
                        p.descendants.discard(d.name)


_orig_assign_ticks = _tsa.TileClockTick.assign_ticks
_install_done = False


def _install_prune():
    global _install_done
    if _install_done:
        return
    _install_done = True

    def assign_ticks_pruned(self, bb_name):
        _prune_blocks(self.ordered_instructions_by_block)
        return _orig_assign_ticks(self, bb_name)

    _tsa.TileClockTick.assign_ticks = assign_ticks_pruned


# ---------------------------------------------------------------- matching


def greedy_pair(support):
    """support: [N, D] bool rows. Pair rows maximizing overlap; [N/2, 2]."""
    N = support.shape[0]
    A = support.astype(np.int32)
    O = A @ A.T
    np.fill_diagonal(O, -1)
    pairs = []
    for _ in range(N // 2):
        idx = int(np.argmax(O))
        i, j = divmod(idx, N)
        pairs.append((i, j))
        O[i, :] = -1
        O[:, i] = -1
        O[j, :] = -1
        O[:, j] = -1
    return np.array(pairs, dtype=np.int64)


def analyze_mask(mask):
    """Returns (in_pairs [128,2], groups [64][4 block ids], sc64 [64,128])."""
    bm = mask.reshape(NB, BLK, NB, BLK).any(axis=(1, 3))  # [out_blk, in_blk]
    in_pairs = greedy_pair(bm.T)
    best = None
    for _ in range(4):
        bmc = bm[:, in_pairs[:, 0]] | bm[:, in_pairs[:, 1]]  # [256, 128]
        out_pairs = greedy_pair(bmc)
        sc32 = bmc[out_pairs[:, 0]] | bmc[out_pairs[:, 1]]
        rp = greedy_pair(sc32)
        sc64 = sc32[rp[:, 0]] | sc32[rp[:, 1]]  # [64, 128]
        groups = np.array(
            [[out_pairs[a][0], out_pairs[a][1], out_pairs[b][0], out_pairs[b][1]]
             for a, b in rp], dtype=np.int64)
        cells = int(sc64.sum())
        if best is None or cells < best[0]:
            best = (cells, in_pairs.copy(), groups, sc64)
        bg = np.zeros((NG, NB), dtype=bool)  # [group, in_blk]
        for g in range(NG):
            bg[g] = bm[groups[g]].any(axis=0)
        in_pairs = greedy_pair(bg.T)
    _, in_pairs, groups, _ = best

    # refine the block->group assignment by hill-climbing on total cells
    bp = bm[:, in_pairs[:, 0]] | bm[:, in_pairs[:, 1]]  # [out_blk, pair]
    groups = groups.copy()
    cnt = np.zeros((NG, NPAIR), dtype=np.int16)
    for g in range(NG):
        cnt[g] = bp[groups[g]].sum(axis=0)
    cells = int((cnt > 0).sum())
    rng = np.random.default_rng(1)
    gi = np.zeros(NB, dtype=np.int64)  # block -> group
    pos = np.zeros(NB, dtype=np.int64)
    for g in range(NG):
        for k in range(4):
            gi[groups[g][k]] = g
            pos[groups[g][k]] = k
    for _ in range(400000):
        u, v = rng.integers(0, NB, 2)
        g1, g2 = gi[u], gi[v]
        if g1 == g2:
            continue
        n1 = cnt[g1] - bp[u] + bp[v]
        n2 = cnt[g2] - bp[v] + bp[u]
        d = (int((n1 > 0).sum()) + int((n2 > 0).sum())
             - int((cnt[g1] > 0).sum()) - int((cnt[g2] > 0).sum()))
        if d <= 0:
            cnt[g1], cnt[g2] = n1, n2
            k1, k2 = pos[u], pos[v]
            groups[g1][k1], groups[g2][k2] = v, u
            gi[u], gi[v] = g2, g1
            pos[u], pos[v] = k2, k1
            cells += d
    sc64 = cnt > 0
    return in_pairs, groups, sc64


def _rebalance_pairs(sc64, set_G):
    """Permute pair layout positions to even out per-set slot loads."""
    rng = np.random.default_rng(0)
    perm = np.arange(NPAIR)
    use_ids = [[np.nonzero(sc64[set_G[s][c]])[0] for c in range(2)]
               for s in range(NSET)]

    def set_cost(p):
        rg = np.zeros(NPAIR, dtype=np.int64)
        rg[p] = np.arange(NPAIR) % 4
        tot = 0
        for s in range(NSET):
            mx = 1
            for c in range(2):
                ids = use_ids[s][c]
                if len(ids):
                    mx = max(mx, int(np.bincount(rg[ids], minlength=4).max()))
            tot += mx
        return tot

    best = set_cost(perm)
    for _ in range(20000):
        i, j = rng.integers(0, NPAIR, 2)
        if i == j or (i % 4) == (j % 4):
            continue
        perm[[i, j]] = perm[[j, i]]
        c = set_cost(perm)
        if c <= best:
            best = c
        else:
            perm[[i, j]] = perm[[j, i]]
    return perm


def build_schedule(sc64):
    counts = sc64.sum(axis=1)
    order = np.argsort(-counts)
    set_G = [[int(order[s * 2 + c]) for c in range(2)] for s in range(NSET)]
    perm = _rebalance_pairs(sc64, set_G)
    pair_pos = np.empty(NPAIR, dtype=np.int64)
    pair_pos[perm] = np.arange(NPAIR)
    slot_lists = []
    for s in range(NSET):
        rows = []
        for r in range(4):
            cols = []
            for c in range(2):
                G = set_G[s][c]
                cs = sorted(int(pair_pos[i]) for i in np.nonzero(sc64[G])[0]
                            if pair_pos[i] % 4 == r)
                cols.append(cs)
            rows.append(cols)
        slot_lists.append(rows)
    return set_G, slot_lists, perm


# ---------------------------------------------------------------- device


def build_sparse(slot_lists, dt=F16):
    _install_prune()
    nc = bacc.Bacc("TRN2", target_bir_lowering=False, debug=False)

    n_sr = np.zeros((NSET, 4), dtype=np.int64)
    for s in range(NSET):
        for r in range(4):
            n_sr[s, r] = sum(max(1, len(slot_lists[s][r][c])) for c in range(2))
    wsz = n_sr.max(axis=1)  # per-set uniform class width (padded)
    maxn = int(wsz.max())
    w_offs = np.zeros(NSET, dtype=np.int64)
    off = 0
    for s in range(NSET):
        w_offs[s] = off
        off += P * int(wsz[s]) * M
    w_words = int(off)

    xT = nc.dram_tensor("xT", [P, KO, TOK], dt, kind="ExternalInput")
    w = nc.dram_tensor("w", [w_words], dt, kind="ExternalInput")
    bias = nc.dram_tensor("bias", [P, NSET], F32, kind="ExternalInput")
    outT = nc.dram_tensor("outT", [NSET, P, TOK], F16, kind="ExternalOutput")

    with tile.TileContext(nc) as tc:
        with (
            tc.tile_pool(name="x_pool", bufs=1) as x_pool,
            tc.tile_pool(name="const", bufs=1) as const_pool,
            tc.tile_pool(name="w_pool", bufs=6) as w_pool,
            tc.tile_pool(name="drain_pool", bufs=4) as drain_pool,
            tc.tile_pool(name="out_pool", bufs=6) as out_pool,
            tc.tile_pool(name="psum", bufs=1, space="PSUM") as psum_pool,
        ):
            XSPLIT = 4
            NXG = KO // XSPLIT
            xts = []
            for g in range(NXG):
                xg = x_pool.tile([P, XSPLIT, TOK], dt, name=f"x{g}", tag=f"x{g}")
                xts.append(xg)

            def emit_x_dma(g):
                nc.sync.dma_start(
                    xts[g][:], xT.ap()[:, g * XSPLIT: (g + 1) * XSPLIT]
                )

            emit_x_dma(0)
            bt = const_pool.tile([P, NSET], F32, name="bt")
            nc.sync.dma_start(bt[:], bias.ap())

            # PE warm-up: ~60 dummy matmuls on sub-array (0,0) keep the
            # tensor engine busy while x streams in, so the HAM clock gate
            # reaches K=8/8 (2.4 GHz) before real work starts.
            zt = const_pool.tile([32, 1024], F16, name="zt")
            nc.vector.memset(zt[:], 0.0)
            ps_warm = psum_pool.tile([P, NCHUNK], F32, tag="ps_0_0",
                                     name="ps_warm")
            for _ in range(60):
                nc.tensor.matmul(ps_warm[0:64, :], zt[:, 0:64],
                                 zt[:, 64:576], start=True, stop=True,
                                 tile_position=(0, 0))

            def emit_w_dma(s, wt):
                nwords = P * int(wsz[s]) * M
                o = int(w_offs[s])
                src = w.ap()[o:o + nwords].rearrange("(p f) -> p f", p=P)
                nc.sync.dma_start(wt[:, :int(wsz[s]) * M], src)

            # prefetch first four sets' weights ahead of the bulk of x
            pre_wt = {}
            for s in range(4):
                wt = w_pool.tile([P, maxn * M], dt, tag="w", name="wt")
                emit_w_dma(s, wt)
                pre_wt[s] = wt
            for g in range(1, NXG):
                emit_x_dma(g)

            for s in range(NSET):
                wt = pre_wt.pop(s)
                for sp in (s + 3, s + 4):
                    if sp < NSET and sp not in pre_wt and sp > s:
                        wtp = w_pool.tile([P, maxn * M], dt, tag="w",
                                          name="wt")
                        emit_w_dma(sp, wtp)
                        pre_wt[sp] = wtp

                ps = [
                    [psum_pool.tile([P, NCHUNK], F32, tag=f"ps_{n}_{r}",
                                    name=f"ps_{n}_{r}")
                     for r in range(4)]
                    for n in range(NT)
                ]

                items = {}
                for r in range(4):
                    for c in range(2):
                        lst = slot_lists[s][r][c]
                        items[(r, c)] = lst if lst else [-1]
                slot_widx = {}
                for r in range(4):
                    k = 0
                    for c in range(2):
                        slot_widx[(r, c)] = k
                        k += len(items[(r, c)])
                nsteps = max(len(items[(r, c)])
                             for r in range(4) for c in range(2))

                for n in range(NT):
                    for st in range(nsteps):
                        for r in range(4):
                            for c in range(2):
                                lst = items[(r, c)]
                                if st >= len(lst):
                                    continue
                                ci = lst[st]
                                wi = slot_widx[(r, c)] + st
                                lhsT = wt[32 * r: 32 * r + 32,
                                          wi * M: (wi + 1) * M]
                                start = st == 0
                                stop = st == len(lst) - 1
                                if ci < 0:
                                    ko, rg = 0, r
                                else:
                                    ko, rg = ci // 4, ci % 4
                                rhs = xts[ko // XSPLIT][
                                    32 * rg: 32 * rg + 32, ko % XSPLIT,
                                    n * NCHUNK: (n + 1) * NCHUNK]
                                nc.tensor.matmul(
                                    ps[n][r][M * c: M * c + M, :], lhsT, rhs,
                                    start=start, stop=stop,
                                    tile_position=(32 * r, M * c),
                                )
                    at = drain_pool.tile([P, NCHUNK], F32, tag="a", name="at")
                    nc.scalar.activation(
                        at[:], ps[n][0][:],
                        mybir.ActivationFunctionType.Identity,
                        bias=bt[:, s: s + 1],
                    )
                    bt2 = drain_pool.tile([P, NCHUNK], F32, tag="b",
                                          name="bt2")
                    nc.scalar.copy(bt2[:], ps[n][3][:])
                    ut = drain_pool.tile([P, NCHUNK], F32, tag="u", name="ut")
                    nc.vector.tensor_tensor(ut[:], ps[n][1][:], at[:],
                                            mybir.AluOpType.add)
                    nc.vector.tensor_tensor(ut[:], ut[:], ps[n][2][:],
                                            mybir.AluOpType.add)
                    ot = out_pool.tile([P, NCHUNK], F16, tag="ot", name="ot")
                    nc.gpsimd.tensor_tensor(ot[:], bt2[:], ut[:],
                                            mybir.AluOpType.add)
                    nc.gpsimd.dma_start(
                        outT.ap()[s, :, n * NCHUNK: (n + 1) * NCHUNK], ot[:]
                    )
    nc.compile()
    return nc, n_sr


def build_dense(dt=F16):
    """Dense fallback: [128,128,512] matmuls, K-contiguous, out-group major."""
    NM = OUT // P
    nc = bacc.Bacc("TRN2", target_bir_lowering=False, debug=False)
    xT = nc.dram_tensor("xT", [P, KO, TOK], dt, kind="ExternalInput")
    w = nc.dram_tensor("w", [NM, P, KO, P], dt, kind="ExternalInput")
    bias = nc.dram_tensor("bias", [P, NM], F32, kind="ExternalInput")
    outT = nc.dram_tensor("outT", [NM, P, TOK], F32, kind="ExternalOutput")

    with tile.TileContext(nc) as tc:
        with (
            tc.tile_pool(name="x_pool", bufs=1) as x_pool,
            tc.tile_pool(name="const", bufs=1) as const_pool,
            tc.tile_pool(name="w_pool", bufs=3) as w_pool,
            tc.tile_pool(name="out_pool", bufs=4) as out_pool,
            tc.tile_pool(name="psum", bufs=2, space="PSUM") as psum_pool,
        ):
            xt = x_pool.tile([P, KO, TOK], dt, name="xt")
            nc.sync.dma_start(xt[:], xT.ap())
            bt = const_pool.tile([P, NM], F32, name="bt")
            nc.sync.dma_start(bt[:], bias.ap())
            for m in range(NM):
                wt = w_pool.tile([P, KO, P], dt, name="wt")
                nc.sync.dma_start(wt[:], w.ap()[m])
                for n in range(NT):
                    psd = psum_pool.tile([P, NCHUNK], F32, name="psd")
                    for ko in range(KO):
                        nc.tensor.matmul(
                            psd[:], wt[:, ko],
                            xt[:, ko, n * NCHUNK: (n + 1) * NCHUNK],
                            start=(ko == 0), stop=(ko == KO - 1),
                        )
                    ot = out_pool.tile([P, NCHUNK], F32, name="ot")
                    nc.scalar.activation(
                        ot[:], psd[:], mybir.ActivationFunctionType.Identity,
                        bias=bt[:, m: m + 1],
                    )
                    nc.sync.dma_start(
                        outT.ap()[m, :, n * NCHUNK: (n + 1) * NCHUNK], ot[:]
                    )
    nc.compile()
    return nc


# ---------------------------------------------------------------- packing


def group_feats(groups, G):
    return np.concatenate([np.arange(b * BLK, (b + 1) * BLK)
                           for b in groups[G]])


def pack_weights(weight, mask, in_pairs, groups, set_G, slot_lists, n_sr):
    wm = weight.astype(np.float32) * mask
    wsz = n_sr.max(axis=1)
    total = int(P * (wsz * M).sum())
    out = np.zeros(total, dtype=np.float32)
    off = 0
    for s in range(NSET):
        nn = int(wsz[s])
        blockbuf = np.zeros((P, nn * M), dtype=np.float32)
        for r in range(4):
            k = 0
            for c in range(2):
                G = set_G[s][c]
                ofeat = group_feats(groups, G)
                lst = slot_lists[s][r][c]
                if not lst:
                    k += 1
                    continue
                for ci in lst:
                    a, b = in_pairs[ci]
                    ifeat = np.concatenate(
                        [np.arange(a * BLK, (a + 1) * BLK),
                         np.arange(b * BLK, (b + 1) * BLK)]
                    )
                    blockbuf[32 * r:32 * r + 32, k * M:(k + 1) * M] = \
                        wm[np.ix_(ofeat, ifeat)].T
                    k += 1
        nw = P * nn * M
        out[off:off + nw] = blockbuf.reshape(-1)
        off += nw
    return out.astype(np.float16)


def pack_x_shard(x_shard, in_pairs):
    src_feat = np.empty((P, KO), dtype=np.int64)
    for i in range(NPAIR):
        a, b = in_pairs[i]
        ko, rg = i // 4, i % 4
        src_feat[rg * 32: rg * 32 + 16, ko] = np.arange(a * BLK, (a + 1) * BLK)
        src_feat[rg * 32 + 16: rg * 32 + 32, ko] = np.arange(b * BLK,
                                                             (b + 1) * BLK)
    xs = x_shard.astype(np.float16)
    xt = xs.T[src_feat.reshape(-1)].reshape(P, KO, TOK)
    return np.ascontiguousarray(xt)


def pack_bias(bias, groups, set_G):
    bp = np.zeros((P, NSET), dtype=np.float32)
    b = bias.astype(np.float32)
    for s in range(NSET):
        for c in range(2):
            bp[M * c: M * c + M, s] = b[group_feats(groups, set_G[s][c])]
    return bp


def out_feat_map(groups, set_G):
    m = np.empty(OUT, dtype=np.int64)
    for s in range(NSET):
        for c in range(2):
            m[s * P + M * c: s * P + M * c + M] = group_feats(
                groups, set_G[s][c])
    return m


# ---------------------------------------------------------------- entry

_CACHE = {}


def _run_sparse(x, weight, bias, mask, plan):
    nc, in_pairs, groups, set_G, slot_lists, n_sr = plan
    w_flat = pack_weights(weight, mask, in_pairs, groups, set_G,
                          slot_lists, n_sr)
    bias_p = pack_bias(bias, groups, set_G)
    B, S = x.shape[0], x.shape[1]
    xf = np.ascontiguousarray(x.reshape(B * S, IN))
    in_maps = []
    for cidx in range(N_CORES):
        xs = xf[cidx * TOK: (cidx + 1) * TOK]
        in_maps.append({"xT": pack_x_shard(xs, in_pairs), "w": w_flat,
                        "bias": bias_p})
    res = bass_utils.run_bass_kernel_spmd(
        nc, in_maps, core_ids=list(range(N_CORES)))
    fmap = out_feat_map(groups, set_G)
    outs = []
    for cidx in range(N_CORES):
        o = res.results[cidx]["outT"].reshape(OUT, TOK).astype(np.float32)
        unperm = np.empty_like(o)
        unperm[fmap] = o
        outs.append(unperm.T)
    full = np.concatenate(outs, axis=0)
    return np.ascontiguousarray(full.reshape(B, S, OUT))


def _run_dense(x, weight, bias, mask, nc):
    NM = OUT // P
    wm = (weight.astype(np.float32) * mask).astype(np.float16)
    w_packed = np.ascontiguousarray(
        wm.T.reshape(KO, P, NM, P).transpose(2, 1, 0, 3))
    bias_p = np.ascontiguousarray(bias.astype(np.float32).reshape(NM, P).T)
    B, S = x.shape[0], x.shape[1]
    xf = np.ascontiguousarray(x.reshape(B * S, IN))
    in_maps = []
    for cidx in range(N_CORES):
        xs = xf[cidx * TOK: (cidx + 1) * TOK].astype(np.float16)
        xp = np.ascontiguousarray(xs.T.reshape(KO, P, TOK).transpose(1, 0, 2))
        in_maps.append({"xT": xp, "w": w_packed, "bias": bias_p})
    res = bass_utils.run_bass_kernel_spmd(
        nc, in_maps, core_ids=list(range(N_CORES)))
    outs = []
    for cidx in range(N_CORES):
        o = res.results[cidx]["outT"].reshape(OUT, TOK)
        outs.append(o.T)
    full = np.concatenate(outs, axis=0)
    return np.ascontiguousarray(full.reshape(B, S, OUT).astype(np.float32))


def kernel(x, weight, bias, mask):
    x = np.asarray(x, dtype=np.float32)
    weight = np.asarray(weight, dtype=np.float32)
    bias = np.asarray(bias, dtype=np.float32)
    mask = np.asarray(mask).astype(bool)
    assert x.shape == (4, 2048, IN) and weight.shape == (OUT, IN)

    key = hash(mask.tobytes())
    if key not in _CACHE:
        in_pairs, groups, sc64 = analyze_mask(mask)
        cells = int(sc64.sum())
        if cells <= SPARSE_MAX_CELLS:
            set_G, slot_lists, perm = build_schedule(sc64)
            in_pairs = in_pairs[perm]
            nc, n_sr = build_sparse(slot_lists)
            _CACHE[key] = ("sparse",
                           (nc, in_pairs, groups, set_G, slot_lists, n_sr))
        else:
            _CACHE[key] = ("dense", build_dense())
    kind, plan = _CACHE[key]
    if kind == "sparse":
        return _run_sparse(x, weight, bias, mask, plan)
    return _run_dense(x, weight, bias, mask, plan)
